# revision 2
# baseline (speedup 1.0000x reference)
"""ConvMultiStepAttention TRN2 Bass kernel.

Per batch (B=16, C=T=S=1024):
  preatt = W @ x           [C,T]
  target = (base + preatt + b) * sqrt(1/2)
  scores = target.T @ enc_top          [T,S]
  attn   = softmax_S(scores)           [T,S]   (output)
  ctx    = enc_combine @ attn.T        [C,T]   (output, as [B,C,T,1])

Sharding: data-parallel over batch, 2 batches per core on 8 cores.
All matmuls run in float32r (TRN2 full-rate fp32 mode, ~12-bit mantissa,
~1e-3 end-to-end rel err). Softmax/epilogues in fp32.
"""
import sys

if "/opt/trn_rl_repo" not in sys.path:
    sys.path.insert(0, "/opt/trn_rl_repo")

import numpy as np

SCALE = np.float32(0.5 ** 0.5)
B, C, T, S = 16, 1024, 1024, 1024
N_CORES = 8
BPC = B // N_CORES          # batches per core
P = 128
KC = C // P                 # 8 partition chunks of the channel dim
TCH = 512                   # t-chunk (matmul moving free dim)
NT = T // TCH               # 2 t-chunks
NJ = TCH // P               # 4 t-row blocks per chunk
NS = 2                      # S halves of 512

_cache = {}


def _build():
    import concourse.bacc as bacc
    import concourse.tile as tile
    from concourse import mybir
    from concourse.masks import make_identity
    from contextlib import ExitStack

    f32 = mybir.dt.float32
    f32r = mybir.dt.float32r
    AF = mybir.ActivationFunctionType
    AX = mybir.AxisListType

    nc = bacc.Bacc("TRN2", target_bir_lowering=False, debug=False)

    x_d = nc.dram_tensor("x", [BPC, C, T], f32, kind="ExternalInput")
    base_d = nc.dram_tensor("base", [BPC, C, T], f32, kind="ExternalInput")
    et_d = nc.dram_tensor("etop", [BPC, C, S], f32, kind="ExternalInput")
    ec_d = nc.dram_tensor("ectT", [BPC, S, C], f32, kind="ExternalInput")
    wt_d = nc.dram_tensor("wT", [C, C], f32, kind="ExternalInput")
    sb_d = nc.dram_tensor("sbias", [P, KC], f32, kind="ExternalInput")
    ctx_d = nc.dram_tensor("ctx", [BPC, C, T], f32, kind="ExternalOutput")
    att_d = nc.dram_tensor("attn", [BPC, T, S], f32, kind="ExternalOutput")

    def as_pkd(ap, p=P):  # [(k p), d] dram view -> [p, k, d]
        return ap.rearrange("(k p) d -> p k d", p=p)

    with tile.TileContext(nc) as tc, ExitStack() as ctx:
        const = ctx.enter_context(tc.tile_pool(name="const", bufs=1))
        wt_pool = ctx.enter_context(tc.tile_pool(name="wt", bufs=1))
        et_pool = ctx.enter_context(tc.tile_pool(name="et", bufs=1))
        ec_pool = ctx.enter_context(tc.tile_pool(name="ec", bufs=1))
        x_pool = ctx.enter_context(tc.tile_pool(name="x", bufs=1))
        tg_pool = ctx.enter_context(tc.tile_pool(name="tg", bufs=1))
        bs_pool = ctx.enter_context(tc.tile_pool(name="bs", bufs=3))
        t1_pool = ctx.enter_context(tc.tile_pool(name="t1", bufs=3))
        row_pool = ctx.enter_context(tc.tile_pool(name="row", bufs=2))
        an_pool = ctx.enter_context(tc.tile_pool(name="an", bufs=2))
        attT_pool = ctx.enter_context(tc.tile_pool(name="attT", bufs=1))
        ctxo_pool = ctx.enter_context(tc.tile_pool(name="ctxo", bufs=1))
        st_pool = ctx.enter_context(tc.tile_pool(name="st", bufs=8))
        ps_pool = ctx.enter_context(tc.tile_pool(name="ps", bufs=4, space="PSUM"))
        tps_pool = ctx.enter_context(tc.tile_pool(name="tps", bufs=2, space="PSUM"))

        ident = const.tile([P, P], f32)
        make_identity(nc, ident)
        ident_r = const.tile([P, P], f32r)
        nc.vector.tensor_copy(ident_r, ident)
        sbias = const.tile([P, KC], f32)
        nc.sync.dma_start(out=sbias, in_=sb_d[:, :])

        wt_sb = wt_pool.tile([P, KC, C], f32r)
        nc.sync.dma_start(out=wt_sb, in_=as_pkd(wt_d[:, :]).bitcast(f32r))

        for b in range(BPC):
            et_sb = et_pool.tile([P, KC, S], f32r, name="et_sb")
            nc.sync.dma_start(out=et_sb, in_=as_pkd(et_d[b]).bitcast(f32r))
            ec_sb = ec_pool.tile([P, KC, C], f32r, name="ec_sb")
            nc.sync.dma_start(out=ec_sb, in_=as_pkd(ec_d[b]).bitcast(f32r))

            for c in range(NT):
                tsl = slice(c * TCH, (c + 1) * TCH)
                x_sb = x_pool.tile([P, KC, TCH], f32r, name="x_sb")
                nc.sync.dma_start(out=x_sb, in_=as_pkd(x_d[b, :, tsl]).bitcast(f32r))

                # ---- phase 1: target = (base + W@x + b) * SCALE ----
                tg_sb = tg_pool.tile([P, KC, TCH], f32r, name="tg_sb")
                for m in range(KC):
                    ps = ps_pool.tile([P, TCH], f32, name="mm_ps")
                    for k in range(KC):
                        nc.tensor.matmul(
                            ps,
                            wt_sb[:, k, m * P:(m + 1) * P],
                            x_sb[:, k, :],
                            start=(k == 0),
                            stop=(k == KC - 1),
                        )
                    bs_sb = bs_pool.tile([P, TCH], f32, name="bs_sb")
                    nc.sync.dma_start(
                        out=bs_sb, in_=base_d[b, m * P:(m + 1) * P, tsl]
                    )
                    t1 = t1_pool.tile([P, TCH], f32, name="t1")
                    nc.vector.tensor_add(t1, ps, bs_sb)
                    nc.scalar.activation(
                        tg_sb[:, m, :], t1, AF.Identity,
                        bias=sbias[:, m:m + 1], scale=float(SCALE),
                    )

                # ---- phase 2: scores + softmax + transpose (pipelined) ----
                # Transpose of row j is traced during row j+1 so the PE never
                # waits on the softmax chain of the row it just produced.
                attT = attT_pool.tile([P, KC, TCH], f32r, name="attT")
                an_tiles = [None] * NJ

                def transpose_row(j):
                    an = an_tiles[j]
                    for sb8 in range(KC):
                        tps = tps_pool.tile([P, P], f32, name="tp_ps")
                        nc.tensor.transpose(
                            tps.bitcast(f32r),
                            an[:, sb8 * P:(sb8 + 1) * P],
                            ident_r,
                        )
                        nc.vector.tensor_copy(
                            attT[:, sb8, j * P:(j + 1) * P], tps
                        )

                for j in range(NJ):
                    row = row_pool.tile([P, S], f32, name="row")
                    for sh in range(NS):
                        ps = ps_pool.tile([P, 512], f32, name="mm_ps")
                        for k in range(KC):
                            nc.tensor.matmul(
                                ps,
                                tg_sb[:, k, j * P:(j + 1) * P],
                                et_sb[:, k, sh * 512:(sh + 1) * 512],
                                start=(k == 0),
                                stop=(k == KC - 1),
                            )
                        nc.vector.tensor_copy(row[:, sh * 512:(sh + 1) * 512], ps)
                    negmax = st_pool.tile([P, 1], f32, name="negmax")
                    nc.vector.reduce_max(negmax, row, axis=AX.X, negate=True)
                    rowsum = st_pool.tile([P, 1], f32, name="rowsum")
                    nc.scalar.activation(
                        row, row, AF.Exp, bias=negmax, scale=1.0, accum_out=rowsum
                    )
                    rinv = st_pool.tile([P, 1], f32, name="rinv")
                    nc.vector.reciprocal(rinv, rowsum)
                    an = an_pool.tile([P, S], f32r, name="an")
                    nc.vector.tensor_scalar_mul(an, row, rinv)
                    r0 = (c * NJ + j) * P
                    nc.sync.dma_start(out=att_d[b, r0:r0 + P, :], in_=an.bitcast(f32))
                    an_tiles[j] = an
                    if j >= 1:
                        transpose_row(j - 1)
                transpose_row(NJ - 1)

                # ---- phase 3: ctx = encC @ attn.T ----
                ctxo = ctxo_pool.tile([P, KC, TCH], f32, name="ctxo")
                for m in range(KC):
                    ps = ps_pool.tile([P, TCH], f32, name="mm_ps")
                    for k in range(KC):
                        nc.tensor.matmul(
                            ps,
                            ec_sb[:, k, m * P:(m + 1) * P],
                            attT[:, k, :],
                            start=(k == 0),
                            stop=(k == KC - 1),
                        )
                    nc.scalar.copy(ctxo[:, m, :], ps)
                nc.sync.dma_start(out=as_pkd(ctx_d[b, :, tsl]), in_=ctxo)

    nc.finalize()
    return nc


def kernel(base_target_emb, x, encoder_out_top, encoder_out_combine, W, b):
    from concourse.bass_utils import run_bass_kernel_spmd

    if "nc" not in _cache:
        _cache["nc"] = _build()
    nc = _cache["nc"]

    x = np.ascontiguousarray(np.asarray(x, np.float32)[:, :, :, 0])
    base = np.ascontiguousarray(np.asarray(base_target_emb, np.float32)[:, :, :, 0])
    et = np.ascontiguousarray(np.asarray(encoder_out_top, np.float32))
    ecT = np.ascontiguousarray(np.asarray(encoder_out_combine, np.float32).transpose(0, 2, 1))
    wT = np.ascontiguousarray(np.asarray(W, np.float32).T)
    sb = np.ascontiguousarray((SCALE * np.asarray(b, np.float32)).reshape(KC, P).T)

    in_maps = []
    for i in range(N_CORES):
        s = slice(i * BPC, (i + 1) * BPC)
        in_maps.append({
            "x": x[s], "base": base[s], "etop": et[s], "ectT": ecT[s],
            "wT": wT, "sbias": sb,
        })

    res = run_bass_kernel_spmd(nc, in_maps, core_ids=list(range(N_CORES)))
    ctx = np.concatenate([r["ctx"] for r in res.results], axis=0)
    attn = np.concatenate([r["attn"] for r in res.results], axis=0)
    return ctx[..., None], attn
